# revision 1
# baseline (speedup 1.0000x reference)
"""Trainium2 Bass kernel for the IWE (image-warped-events) problem.

Full inputs in, full outputs out. Data-parallel over (batch, half) across 8
NeuronCores; each core computes a partial IWE grid over its events plus the
avg_flow channels; host sums the two partial IWEs per batch.

Per-core pipeline, per 500-event chunk (4 blocks x 125 events on partitions):
  - one-hot(y) via rank-2 matmul (y_e - y) into PSUM + is_equal(0)
  - flow gather: onehotY @ [fy|fx] rows; x-select via product with onehot(x)
    and a blocked free-dim reduction
  - warped coords; bilinear "hat" weight rows hat(t)=relu(1-|t|) on iota grids
    (corner weights + OOB masking in one shot)
  - scatter-add via PSUM-accumulated matmuls: SUM = hatY^T@hatX,
    DIFF = (hatY*sgn)^T@hatX;  pos=(SUM+DIFF)/2, neg=(SUM-DIFF)/2
"""
import numpy as np

H, W = 128, 128
NCORES = 8
CHUNK = 500                            # events per chunk
PBLK = 125                             # partition rows used for events
NBLK = 4                               # blocks per chunk (PBLK*NBLK = CHUNK)

_COMPILED = {}


def _build(nchunks, use_hw_loop=True, unroll=4, passes=1):
    import concourse.bass as bass
    import concourse.bacc as bacc
    import concourse.mybir as mybir
    from concourse.tile import TileContext

    fp32 = mybir.dt.float32
    bf16 = mybir.dt.bfloat16
    int32 = mybir.dt.int32
    Alu = mybir.AluOpType
    E = nchunks * CHUNK

    nc = bacc.Bacc("TRN2", target_bir_lowering=False, debug=False,
                   num_devices=NCORES)

    ev = nc.dram_tensor("ev", [E, 4], fp32, kind="ExternalInput").ap()
    pol = nc.dram_tensor("pol", [E, 2], fp32, kind="ExternalInput").ap()
    flow = nc.dram_tensor("flow", [2, H, W], fp32, kind="ExternalInput").ap()
    emask = nc.dram_tensor("emask", [H, W], fp32, kind="ExternalInput").ap()
    out = nc.dram_tensor("out", [4, H, W], fp32, kind="ExternalOutput").ap()

    # event id e = c*CHUNK + p*NBLK + n  (chunk c, partition p, block n)
    ev_v = ev.rearrange("(c p n) f -> p c (n f)", c=nchunks, p=PBLK, n=NBLK)
    evy_v = ev.rearrange("(c p n) f -> n c p f", c=nchunks, p=PBLK, n=NBLK)
    pol_v = pol.rearrange("(c p n) f -> p c (n f)", c=nchunks, p=PBLK, n=NBLK)

    with TileContext(nc) as tc:
        with tc.tile_pool(name="const", bufs=1) as cpool, \
             tc.tile_pool(name="work", bufs=2) as wpool, \
             tc.tile_pool(name="ppool", bufs=1, space="PSUM") as ppool:

            # ---------------- constants ----------------
            iotai = cpool.tile([128, 128], int32)
            nc.gpsimd.iota(iotai[:], pattern=[[1, 128]], base=0, channel_multiplier=0)
            iota32 = cpool.tile([128, 128], fp32)
            nc.vector.tensor_copy(out=iota32[:], in_=iotai[:])
            # dy matmul weights: lhsT_j [16,128]: row j = ones, row 4 = -iota
            pidi = cpool.tile([16, 1], int32)
            nc.gpsimd.iota(pidi[:], pattern=[[1, 1]], base=0, channel_multiplier=1)
            pidf = cpool.tile([16, 1], fp32)
            nc.vector.tensor_copy(out=pidf[:], in_=pidi[:])
            neq4 = cpool.tile([16, 1], fp32)
            nc.vector.tensor_scalar(out=neq4[:], in0=pidf[:], scalar1=4.0,
                                    scalar2=-1.0, op0=Alu.is_equal, op1=Alu.mult)
            dylhs = []
            for j in range(NBLK):
                eqj = cpool.tile([16, 1], fp32, tag=f"eq{j}")
                nc.vector.tensor_scalar(out=eqj[:], in0=pidf[:], scalar1=float(j),
                                        scalar2=None, op0=Alu.is_equal)
                lt = cpool.tile([16, 128], bf16, tag=f"dylhs{j}")
                nc.vector.scalar_tensor_tensor(
                    out=lt[:], in0=iota32[0:16, :], scalar=neq4[:],
                    in1=eqj[:].broadcast_to([16, 128]), op0=Alu.mult, op1=Alu.add)
                dylhs.append(lt)
            # y-row staging: rows 0..3 get per-chunk y rows, rows 4..15 stay 1.0
            yb16 = cpool.tile([16, 128], bf16)
            nc.vector.memset(yb16[:], 1.0)

            # flow rows bf16: [y, 0:128]=flow[1] (fy), [y, 128:256]=flow[0] (fx)
            flow32 = cpool.tile([128, 256], fp32)
            nc.sync.dma_start(out=flow32[:, 0:128], in_=flow[1])
            nc.sync.dma_start(out=flow32[:, 128:256], in_=flow[0])
            flowcat = cpool.tile([128, 256], bf16)
            nc.vector.tensor_copy(out=flowcat[:], in_=flow32[:])

            # ---------------- psum tiles ----------------
            p_dy = ppool.tile([128, CHUNK], fp32, tag="dy")        # [y, e]
            p_t1 = ppool.tile([PBLK, NBLK * 256], fp32, tag="t1")  # [e, blk*(fy|fx)]
            p_gs = ppool.tile([128, 128], fp32, tag="gsum")
            p_gd = ppool.tile([128, 128], fp32, tag="gdiff")
            nc.vector.memset(p_gs[:], 0.0)
            nc.vector.memset(p_gd[:], 0.0)

            def body(i):
                evt = wpool.tile([PBLK, NBLK * 4], fp32, tag="evt")
                polt = wpool.tile([PBLK, NBLK * 2], fp32, tag="polt")
                nc.sync.dma_start(out=evt[:], in_=ev_v[:, bass.ds(i, 1), :])
                nc.sync.dma_start(out=polt[:], in_=pol_v[:, bass.ds(i, 1), :])
                ev3 = evt[:].rearrange("p (n f) -> p n f", f=4)
                ts4 = ev3[:, :, 0]
                y4 = ev3[:, :, 1]
                x4 = ev3[:, :, 2]
                pol0 = polt[:].rearrange("p (n f) -> p n f", f=2)[:, :, 0]

                u4 = wpool.tile([PBLK, NBLK], fp32, tag="u4")
                nc.vector.tensor_scalar(out=u4[:], in0=ts4, scalar1=1.0,
                                        scalar2=-1.0, op0=Alu.subtract, op1=Alu.mult)
                sgn4 = wpool.tile([PBLK, NBLK], bf16, tag="sgn4")
                nc.vector.tensor_scalar(out=sgn4[:], in0=pol0, scalar1=2.0,
                                        scalar2=1.0, op0=Alu.mult, op1=Alu.subtract)
                # y rows via strided DMA: yrow4[n, p] = y of event (c,p,n)
                yrow4 = wpool.tile([4, PBLK], fp32, tag="yrow4")
                nc.sync.dma_start(out=yrow4[:], in_=evy_v[:, bass.ds(i, 1), :, 1])
                nc.vector.tensor_copy(out=yb16[0:4, :PBLK], in_=yrow4[:])

                # dy[y, e] = y_e - y  (4 matmuls into p_dy column blocks)
                for j in range(NBLK):
                    nc.tensor.matmul(
                        out=p_dy[:, j * PBLK:(j + 1) * PBLK],
                        lhsT=dylhs[j][:],
                        rhs=yb16[:, :PBLK],
                        start=True, stop=True)

                onehotY = wpool.tile([128, CHUNK], bf16, tag="ohY")
                nc.vector.tensor_scalar(out=onehotY[:], in0=p_dy[:], scalar1=0.0,
                                        scalar2=None, op0=Alu.is_equal)

                # flow row select: T1[e, fy|fx] per block
                for j in range(NBLK):
                    nc.tensor.matmul(
                        out=p_t1[:, j * 256:(j + 1) * 256],
                        lhsT=onehotY[:, j * PBLK:(j + 1) * PBLK],
                        rhs=flowcat[:],
                        start=True, stop=True)

                onehotX = wpool.tile([PBLK, NBLK * 128], fp32, tag="ohX")
                iota_rep = iota32[:PBLK, :].unsqueeze(1).broadcast_to(
                    [PBLK, NBLK, 128])
                x4_bc = x4.unsqueeze(2).broadcast_to([PBLK, NBLK, 128])
                nc.vector.tensor_tensor(
                    out=onehotX[:].rearrange("p (n f) -> p n f", f=128),
                    in0=iota_rep, in1=x4_bc, op=Alu.is_equal)

                # select: prod = T1 * onehotX (repeated over 2 ch), blocked reduce
                prod = wpool.tile([PBLK, NBLK * 256], bf16, tag="prod")
                ohx_rep = onehotX[:].rearrange(
                    "p (n f) -> p n f", f=128).unsqueeze(2).broadcast_to(
                    [PBLK, NBLK, 2, 128])
                nc.vector.tensor_tensor(
                    out=prod[:].rearrange("p (n c f) -> p n c f", c=2, f=128),
                    in0=p_t1[:].rearrange("p (n c f) -> p n c f", c=2, f=128),
                    in1=ohx_rep, op=Alu.mult)
                sel = wpool.tile([PBLK, NBLK * 2], fp32, tag="sel")
                nc.vector.tensor_reduce(
                    out=sel[:],
                    in_=prod[:].rearrange("p (n f) -> p n f", f=128),
                    axis=mybir.AxisListType.X, op=Alu.add)
                sel3 = sel[:].rearrange("p (n f) -> p n f", f=2)
                fy4 = sel3[:, :, 0]
                fx4 = sel3[:, :, 1]

                # warped coords + clamp
                wy4 = wpool.tile([PBLK, NBLK], fp32, tag="wy4")
                wx4 = wpool.tile([PBLK, NBLK], fp32, tag="wx4")
                nc.vector.tensor_tensor(out=wy4[:], in0=u4[:], in1=fy4, op=Alu.mult)
                nc.vector.tensor_tensor(out=wy4[:], in0=wy4[:], in1=y4, op=Alu.add)
                nc.vector.tensor_scalar(out=wy4[:], in0=wy4[:], scalar1=130.0,
                                        scalar2=-2.0, op0=Alu.min, op1=Alu.max)
                nc.vector.tensor_tensor(out=wx4[:], in0=u4[:], in1=fx4, op=Alu.mult)
                nc.vector.tensor_tensor(out=wx4[:], in0=wx4[:], in1=x4, op=Alu.add)
                nc.vector.tensor_scalar(out=wx4[:], in0=wx4[:], scalar1=130.0,
                                        scalar2=-2.0, op0=Alu.min, op1=Alu.max)

                # hats: t = w - iota ; m = |t| ; hat = -(min(m,1) - 1)
                def hat_from(w4, tag):
                    t = wpool.tile([PBLK, NBLK * 128], fp32, tag=tag + "_t")
                    w_bc = w4.unsqueeze(2).broadcast_to([PBLK, NBLK, 128])
                    nc.vector.scalar_tensor_tensor(
                        out=t[:].rearrange("p (n f) -> p n f", f=128),
                        in0=w_bc, scalar=0.0, in1=iota_rep,
                        op0=Alu.add, op1=Alu.subtract)
                    tneg = wpool.tile([PBLK, NBLK * 128], fp32, tag=tag + "_tn")
                    nc.vector.tensor_scalar_mul(out=tneg[:], in0=t[:], scalar1=-1.0)
                    m = wpool.tile([PBLK, NBLK * 128], bf16, tag=tag + "_m")
                    nc.vector.tensor_tensor(out=m[:], in0=t[:], in1=tneg[:],
                                            op=Alu.max)
                    # negated hat: min(|t|,1) - 1 = -relu(1-|t|)
                    h = wpool.tile([PBLK, NBLK * 128], bf16, tag=tag + "_h")
                    nc.vector.tensor_scalar(out=h[:], in0=m[:], scalar1=1.0,
                                            scalar2=1.0, op0=Alu.min,
                                            op1=Alu.subtract)
                    return h

                hatY = hat_from(wy4, "hy")
                hatX = hat_from(wx4, "hx")
                hatYs = wpool.tile([PBLK, NBLK * 128], bf16, tag="hys")
                sgn_bc = sgn4[:].unsqueeze(2).broadcast_to([PBLK, NBLK, 128])
                nc.vector.tensor_tensor(
                    out=hatYs[:].rearrange("p (n f) -> p n f", f=128),
                    in0=hatY[:].rearrange("p (n f) -> p n f", f=128),
                    in1=sgn_bc, op=Alu.mult)

                for j in range(NBLK):
                    sl = slice(j * 128, (j + 1) * 128)
                    nc.tensor.matmul(out=p_gs[:], lhsT=hatY[:, sl],
                                     rhs=hatX[:, sl], start=False, stop=False)
                    nc.tensor.matmul(out=p_gd[:], lhsT=hatYs[:, sl],
                                     rhs=hatX[:, sl], start=False, stop=False)

            if use_hw_loop:
                for _ in range(passes):
                    tc.For_i_unrolled(0, nchunks, 1, body, max_unroll=unroll)
            else:
                for i in range(nchunks):
                    body(i)

            # ---------------- finalize ----------------
            res = cpool.tile([128, 128 * 4], fp32)
            gd_sb = cpool.tile([128, 128], fp32)
            nc.vector.tensor_copy(out=gd_sb[:], in_=p_gd[:])
            nc.vector.tensor_tensor(out=res[:, 0:128], in0=p_gs[:], in1=gd_sb[:],
                                    op=Alu.add)
            nc.vector.tensor_scalar_mul(out=res[:, 0:128], in0=res[:, 0:128],
                                        scalar1=0.5)
            nc.vector.tensor_tensor(out=res[:, 128:256], in0=p_gs[:], in1=gd_sb[:],
                                    op=Alu.subtract)
            nc.vector.tensor_scalar_mul(out=res[:, 128:256], in0=res[:, 128:256],
                                        scalar1=0.5)
            maskt = cpool.tile([128, 128], fp32)
            nc.sync.dma_start(out=maskt[:], in_=emask[:, :])
            inv = 1.0 / (1.0 + 1e-9)
            nc.vector.scalar_tensor_tensor(out=res[:, 256:384],
                                           in0=flow32[:, 128:256], scalar=inv,
                                           in1=maskt[:], op0=Alu.mult, op1=Alu.mult)
            nc.vector.scalar_tensor_tensor(out=res[:, 384:512],
                                           in0=flow32[:, 0:128], scalar=inv,
                                           in1=maskt[:], op0=Alu.mult, op1=Alu.mult)
            for ch in range(4):
                nc.sync.dma_start(out=out[ch], in_=res[:, ch * 128:(ch + 1) * 128])

    nc.compile()
    return nc


def _run(nc, flow, event_list, pol_mask, event_mask):
    """flow [B,2,H,W], event_list [B,N,4], pol [B,N,2], emask [B,1,H,W]."""
    from concourse.bass_utils import run_bass_kernel_spmd

    Bb, Nn = event_list.shape[0], event_list.shape[1]
    half = Nn // 2
    in_maps = []
    for c in range(NCORES):
        b, h = c // 2, c % 2
        sl = slice(h * half, (h + 1) * half)
        in_maps.append({
            "ev": np.ascontiguousarray(event_list[b, sl, :], np.float32),
            "pol": np.ascontiguousarray(pol_mask[b, sl, :], np.float32),
            "flow": np.ascontiguousarray(flow[b], np.float32),
            "emask": np.ascontiguousarray(event_mask[b, 0], np.float32),
        })
    res = run_bass_kernel_spmd(nc, in_maps, list(range(NCORES)))
    out = np.zeros((Bb, 4, H, W), np.float32)
    for c in range(NCORES):
        b = c // 2
        r = res.results[c]["out"]
        out[b, 0:2] += r[0:2]
        if c % 2 == 0:
            out[b, 2:4] = r[2:4]
    return out


def kernel(flow, event_list, pol_mask, event_mask):
    flow = np.asarray(flow, np.float32)
    event_list = np.asarray(event_list, np.float32)
    pol_mask = np.asarray(pol_mask, np.float32)
    event_mask = np.asarray(event_mask, np.float32)
    nchunks = event_list.shape[0] * event_list.shape[1] // NCORES // CHUNK
    key = ("nc", nchunks)
    if key not in _COMPILED:
        _COMPILED[key] = _build(nchunks)
    return _run(_COMPILED[key], flow, event_list, pol_mask, event_mask)



# revision 2
# speedup vs baseline: 1.3919x; 1.3919x over previous
"""Trainium2 Bass kernel for the IWE (image-warped-events) problem — v2.

Full inputs in, full outputs out. Data-parallel over (batch, half) across 8
NeuronCores. Host splits each core's 500k events by polarity into two padded
streams (scatter target = pos or neg grid picked per stream, no sign math on
device), packs them in a DMA-friendly superchunk layout, and sums nothing:
each core owns its whole grid; host only stitches batch halves.

Per-core pipeline, per 500-event chunk (125 partitions x 4 blocks):
  - transpose y -> [4,125] (PE array), build dy[bin,e]=y_e-bin via K=1
    accumulating matmuls, onehotY = is_equal(dy,0)
  - flow gather: onehotY^T @ [fy|fx] -> T1[e,256]; x-select by onehot(x)
    product + blocked free-dim reduction
  - my = (1-ts)*fy, mx likewise; staging [my,y,1,mx,x,1] per block,
    one PE transpose; dy2[e,bin] = w_e - bin via K=3 matmuls per block/side
  - hats: y-side Abs+Relu(1-|.|) on scalar engine; x-side negated hat via
    fused abs_max/min on vector engine; scatter = PSUM-accumulated matmuls
    hatY^T @ (-hatX) into the stream's grid; finalize negates.
"""
import numpy as np

H, W = 128, 128
NCORES = 8
CHUNK = 500                            # events per chunk
PBLK = 125                             # partition rows used for events
NBLK = 4                               # blocks per chunk (PBLK*NBLK = CHUNK)
SCH = 8                                # chunks per superchunk (one DMA)
NPC_DEFAULT = 504                      # chunks per polarity stream (>= 252k ev)

_COMPILED = {}


def _build(nchunks, passes=1, npc=NPC_DEFAULT, sch=SCH):
    import concourse.bass as bass
    import concourse.bacc as bacc
    import concourse.mybir as mybir
    from concourse.tile import TileContext

    fp32 = mybir.dt.float32
    fp16 = mybir.dt.float16
    int32 = mybir.dt.int32
    Alu = mybir.AluOpType

    assert npc % sch == 0
    nsc = npc // sch                   # superchunks per stream

    nc = bacc.Bacc("TRN2", target_bir_lowering=False, debug=False,
                   num_devices=NCORES)

    # host-packed events: [2 streams * nsc, 125, sch*16] fully contiguous
    evp = nc.dram_tensor("evp", [2 * nsc, PBLK, sch * 16], fp32,
                         kind="ExternalInput").ap()
    flow = nc.dram_tensor("flow", [2, H, W], fp32, kind="ExternalInput").ap()
    emask = nc.dram_tensor("emask", [H, W], fp32, kind="ExternalInput").ap()
    out = nc.dram_tensor("out", [4, H, W], fp32, kind="ExternalOutput").ap()

    evp_v = evp.rearrange("s p f -> p s f")

    with TileContext(nc) as tc:
        with tc.tile_pool(name="const", bufs=1) as cpool, \
             tc.tile_pool(name="work", bufs=4) as wpool, \
             tc.tile_pool(name="ppool", bufs=1, space="PSUM") as ppool:

            # ---------------- constants ----------------
            iotai = cpool.tile([128, 128], int32)
            nc.gpsimd.iota(iotai[:], pattern=[[1, 128]], base=0,
                           channel_multiplier=0)
            iota32 = cpool.tile([128, 128], fp32)
            nc.vector.tensor_copy(out=iota32[:], in_=iotai[:])
            pidx = cpool.tile([128, 1], int32)
            nc.gpsimd.iota(pidx[:], pattern=[[1, 1]], base=0,
                           channel_multiplier=1)
            ident = cpool.tile([128, 128], fp32)
            nc.vector.tensor_tensor(out=ident[:], in0=iotai[:],
                                    in1=pidx[:].broadcast_to([128, 128]),
                                    op=Alu.is_equal)
            pidxf = cpool.tile([128, 1], fp32)
            nc.vector.tensor_copy(out=pidxf[:], in_=pidx[:])

            def mask_tile(nrows, ones_rows, iota_row, tag):
                # [nrows,128] fp16: selected rows 1.0, iota_row = -iota, rest 0
                ind = cpool.tile([nrows, 1], fp32, tag=tag + "_i")
                tmp = cpool.tile([nrows, 1], fp32, tag=tag + "_t")
                nc.vector.memset(ind[:], 0.0)
                for r in ones_rows:
                    nc.vector.tensor_scalar(out=tmp[:], in0=pidxf[0:nrows],
                                            scalar1=float(r), scalar2=None,
                                            op0=Alu.is_equal)
                    nc.vector.tensor_tensor(out=ind[:], in0=ind[:],
                                            in1=tmp[:], op=Alu.add)
                neg = cpool.tile([nrows, 1], fp32, tag=tag + "_n")
                nc.vector.tensor_scalar(out=neg[:], in0=pidxf[0:nrows],
                                        scalar1=float(iota_row), scalar2=-1.0,
                                        op0=Alu.is_equal, op1=Alu.mult)
                t = cpool.tile([nrows, 128], fp16, tag=tag)
                nc.vector.scalar_tensor_tensor(
                    out=t[:], in0=iota32[0:nrows, :], scalar=neg[:],
                    in1=ind[:].broadcast_to([nrows, 128]),
                    op0=Alu.mult, op1=Alu.add)
                return t

            # dy lhsT masks: dylhs[j] [5,128]: row j = 1, row 4 = -iota
            dylhs = [mask_tile(5, [j], 4, f"dylhs{j}") for j in range(NBLK)]
            # dy2 rhs masks per block [24,256]: cols 0:128 y-side,
            # 128:256 x-side; staging field order per block: [my,y,1,mx,x,1]
            d2rhs = []
            for j in range(NBLK):
                hy_m = mask_tile(24, [6 * j, 6 * j + 1], 6 * j + 2,
                                 f"d2y{j}")
                hx_m = mask_tile(24, [6 * j + 3, 6 * j + 4], 6 * j + 5,
                                 f"d2x{j}")
                m = cpool.tile([24, 256], fp16, tag=f"d2m{j}")
                nc.vector.tensor_copy(out=m[:, 0:128], in_=hy_m[:])
                nc.vector.tensor_copy(out=m[:, 128:256], in_=hx_m[:])
                d2rhs.append(m)

            # flow rows: flow32 natural order [flow0|flow1];
            # flowcat fp16 [fy|fx] = [flow1|flow0]
            flow32 = cpool.tile([128, 256], fp32)
            nc.sync.dma_start(out=flow32[:, 0:128], in_=flow[0])
            nc.sync.dma_start(out=flow32[:, 128:256], in_=flow[1])
            flowcat = cpool.tile([128, 256], fp16)
            nc.vector.tensor_copy(out=flowcat[:, 0:128], in_=flow32[:, 128:256])
            nc.vector.tensor_copy(out=flowcat[:, 128:256], in_=flow32[:, 0:128])

            iota_rep = iota32[:PBLK, :].unsqueeze(1).broadcast_to(
                [PBLK, NBLK, 128])

            # ---------------- psum tiles ----------------
            p_tr1 = ppool.tile([5, PBLK], fp32, tag="tr1")       # transpose1
            p_tr2 = ppool.tile([24, PBLK], fp32, tag="tr2")      # transpose2
            p_dy = ppool.tile([128, CHUNK], fp32, tag="dy")      # [bin, e]
            p_t1 = ppool.tile([PBLK, NBLK * 256], fp32, tag="t1")
            p_d2 = ppool.tile([PBLK, NBLK * 256], fp32, tag="d2")
            p_acc = ppool.tile([128, 128], fp32, tag="acc")
            res = cpool.tile([128, 128 * 4], fp32)

            def phase_a(evt16):
                ev3 = evt16.rearrange("p (n f) -> p n f", f=4)
                ts4 = ev3[:, :, 0]
                y4 = ev3[:, :, 1]
                x4 = ev3[:, :, 2]

                u4 = wpool.tile([PBLK, NBLK], fp32, tag="u4")
                nc.vector.tensor_scalar(out=u4[:], in0=ts4, scalar1=-1.0,
                                        scalar2=1.0, op0=Alu.mult, op1=Alu.add)

                # transpose1: [y|1] -> [5, 125]; st_y5 fp16
                stg_y = wpool.tile([PBLK, 5], fp32, tag="stg_y")
                nc.gpsimd.tensor_copy(out=stg_y[:, 0:4], in_=y4)
                nc.gpsimd.memset(stg_y[:, 4:5], 1.0)
                nc.tensor.transpose(p_tr1[:], stg_y[:],
                                    ident[0:PBLK, 0:PBLK])
                st_y5 = wpool.tile([5, PBLK], fp16, tag="st_y5")
                nc.scalar.copy(out=st_y5[:], in_=p_tr1[:])

                # dy[bin, e] = y_e - bin (K=5, mask lhsT per block)
                for j in range(NBLK):
                    nc.tensor.matmul(
                        out=p_dy[:, j * PBLK:(j + 1) * PBLK],
                        lhsT=dylhs[j][:], rhs=st_y5[:],
                        start=True, stop=True)

                onehotY = wpool.tile([128, CHUNK], fp16, tag="ohY")
                nc.vector.tensor_scalar(out=onehotY[:], in0=p_dy[:],
                                        scalar1=0.0, scalar2=None,
                                        op0=Alu.is_equal)

                # flow row gather: T1[e, fy|fx] per block
                for j in range(NBLK):
                    nc.tensor.matmul(
                        out=p_t1[:, j * 256:(j + 1) * 256],
                        lhsT=onehotY[:, j * PBLK:(j + 1) * PBLK],
                        rhs=flowcat[:], start=True, stop=True)

                # x-select: onehot(x) product + blocked reduce
                ohX = wpool.tile([PBLK, NBLK * 128], fp16, tag="ohX")
                x4_bc = x4.unsqueeze(2).broadcast_to([PBLK, NBLK, 128])
                nc.vector.tensor_tensor(
                    out=ohX[:].rearrange("p (n f) -> p n f", f=128),
                    in0=iota_rep, in1=x4_bc, op=Alu.is_equal)
                prod = wpool.tile([PBLK, NBLK * 256], fp16, tag="prod")
                ohx_rep = ohX[:].rearrange(
                    "p (n f) -> p n f", f=128).unsqueeze(2).broadcast_to(
                    [PBLK, NBLK, 2, 128])
                nc.vector.tensor_tensor(
                    out=prod[:].rearrange("p (n c f) -> p n c f", c=2, f=128),
                    in0=p_t1[:].rearrange("p (n c f) -> p n c f", c=2, f=128),
                    in1=ohx_rep, op=Alu.mult)
                sel = wpool.tile([PBLK, NBLK * 2], fp32, tag="sel")
                nc.vector.tensor_reduce(
                    out=sel[:],
                    in_=prod[:].rearrange("p (n f) -> p n f", f=128),
                    axis=mybir.AxisListType.X, op=Alu.add)
                sel3 = sel[:].rearrange("p (n f) -> p n f", f=2)

                # staging [my, y, 1, mx, x, 1] per block (feeds phase_b)
                stg = wpool.tile([PBLK, NBLK * 6], fp32, tag="stg")
                stg3 = stg[:].rearrange("p (n k) -> p n k", k=6)
                nc.gpsimd.tensor_copy(out=stg3[:, :, 1], in_=y4)
                nc.gpsimd.tensor_copy(out=stg3[:, :, 4], in_=x4)
                ones_v = stg[:].rearrange("p (a b) -> p a b", b=3)[:, :, 2]
                nc.gpsimd.memset(ones_v, 1.0)
                nc.vector.tensor_tensor(out=stg3[:, :, 0], in0=u4[:],
                                        in1=sel3[:, :, 0], op=Alu.mult)
                nc.vector.tensor_tensor(out=stg3[:, :, 3], in0=u4[:],
                                        in1=sel3[:, :, 1], op=Alu.mult)
                return stg

            def phase_b(stg, acc, start=False, stop=False):
                nc.tensor.transpose(p_tr2[:], stg[:],
                                    ident[0:PBLK, 0:PBLK])
                st_w = wpool.tile([24, PBLK], fp16, tag="st_w")
                nc.scalar.copy(out=st_w[:], in_=p_tr2[:])

                # dy2[e, (y-bin | x-bin)] per block (K=24, mask rhs)
                for j in range(NBLK):
                    nc.tensor.matmul(
                        out=p_d2[:, j * 256:(j + 1) * 256],
                        lhsT=st_w[:], rhs=d2rhs[j][:],
                        start=True, stop=True)

                # negated hats both sides: h = min(|dy2|,1) - 1
                ad = wpool.tile([PBLK, NBLK * 256], fp16, tag="ad")
                nc.scalar.activation(out=ad[:], in_=p_d2[:],
                                     func=mybir.ActivationFunctionType.Abs)
                hn = wpool.tile([PBLK, NBLK * 256], fp16, tag="hn")
                nc.vector.tensor_scalar(out=hn[:], in0=ad[:], scalar1=1.0,
                                        scalar2=1.0, op0=Alu.min,
                                        op1=Alu.subtract)

                # scatter: acc += hatY (x) hatX  ((-a)(-b) = ab)
                for j in range(NBLK):
                    nc.tensor.matmul(
                        out=acc,
                        lhsT=hn[:, j * 256:j * 256 + 128],
                        rhs=hn[:, j * 256 + 128:(j + 1) * 256],
                        start=(start and j == 0),
                        stop=(stop and j == NBLK - 1))

            def sc_body(i, acc, first=False, last=False):
                evsc = wpool.tile([PBLK, sch * 16], fp32, tag="evsc")
                nc.sync.dma_start(out=evsc[:], in_=evp_v[:, bass.ds(i, 1), :])
                st = phase_a(evsc[:, 0:16])
                for k in range(1, sch):
                    st_next = phase_a(evsc[:, k * 16:(k + 1) * 16])
                    phase_b(st, acc, start=(first and k == 1))
                    st = st_next
                phase_b(st, acc, stop=last)

            for _ in range(passes):
                for s in range(2):
                    lo = s * nsc
                    sc_body(lo, p_acc[:], first=True, last=(nsc == 1))
                    if nsc > 2:
                        tc.For_i_unrolled(lo + 1, lo + nsc - 1, 1,
                                          lambda i: sc_body(i, p_acc[:]),
                                          max_unroll=1)
                    if nsc > 1:
                        sc_body(lo + nsc - 1, p_acc[:], last=True)
                    nc.vector.tensor_copy(out=res[:, s * 128:(s + 1) * 128],
                                          in_=p_acc[:])

            # ---------------- finalize ----------------
            maskt = cpool.tile([128, 128], fp32)
            nc.sync.dma_start(out=maskt[:], in_=emask[:, :])
            inv = 1.0 / (1.0 + 1e-9)
            nc.vector.scalar_tensor_tensor(out=res[:, 256:384],
                                           in0=flow32[:, 0:128], scalar=inv,
                                           in1=maskt[:], op0=Alu.mult,
                                           op1=Alu.mult)
            nc.vector.scalar_tensor_tensor(out=res[:, 384:512],
                                           in0=flow32[:, 128:256], scalar=inv,
                                           in1=maskt[:], op0=Alu.mult,
                                           op1=Alu.mult)
            for ch in range(4):
                nc.sync.dma_start(out=out[ch],
                                  in_=res[:, ch * 128:(ch + 1) * 128])

    nc.compile()
    return nc


def _pack_stream(ev, npc, sch):
    """ev [n, 4] one polarity's events -> [npc//sch, 125, sch*16] fp32."""
    total = npc * CHUNK
    pad = np.zeros((total - ev.shape[0], 4), np.float32)
    if pad.shape[0]:
        pad[:, 0] = 1.0
        pad[:, 1] = -512.0
        pad[:, 2] = -512.0
        ev = np.concatenate([ev, pad], 0)
    nsc = npc // sch
    a = ev.reshape(nsc, sch, PBLK, NBLK * 4)
    return np.ascontiguousarray(a.transpose(0, 2, 1, 3)).reshape(
        nsc, PBLK, sch * 16)


def _make_in_maps(flow, event_list, pol_mask, event_mask, npc=NPC_DEFAULT,
                  sch=SCH):
    half = event_list.shape[1] // 2
    in_maps = []
    for c in range(NCORES):
        b, h = c // 2, c % 2
        ev = event_list[b, h * half:(h + 1) * half, :]
        pos = pol_mask[b, h * half:(h + 1) * half, 0] == 1.0
        packed = np.concatenate([
            _pack_stream(np.ascontiguousarray(ev[pos], np.float32), npc, sch),
            _pack_stream(np.ascontiguousarray(ev[~pos], np.float32), npc, sch),
        ], 0)
        in_maps.append({
            "evp": packed,
            "flow": np.ascontiguousarray(flow[b], np.float32),
            "emask": np.ascontiguousarray(event_mask[b, 0], np.float32),
        })
    return in_maps


def _run(nc, flow, event_list, pol_mask, event_mask, npc=NPC_DEFAULT, sch=SCH):
    from concourse.bass_utils import run_bass_kernel_spmd

    Bb = event_list.shape[0]
    in_maps = _make_in_maps(flow, event_list, pol_mask, event_mask, npc, sch)
    res = run_bass_kernel_spmd(nc, in_maps, list(range(NCORES)))
    out = np.zeros((Bb, 4, H, W), np.float32)
    for c in range(NCORES):
        b = c // 2
        r = res.results[c]["out"]
        out[b, 0:2] += r[0:2]
        if c % 2 == 0:
            out[b, 2:4] = r[2:4]
    return out


def kernel(flow, event_list, pol_mask, event_mask):
    flow = np.asarray(flow, np.float32)
    event_list = np.asarray(event_list, np.float32)
    pol_mask = np.asarray(pol_mask, np.float32)
    event_mask = np.asarray(event_mask, np.float32)
    nchunks = event_list.shape[0] * event_list.shape[1] // NCORES // CHUNK
    # stream capacity: grow npc if a polarity stream overflows the default
    half = event_list.shape[1] // 2
    maxcnt = 0
    for b in range(event_list.shape[0]):
        for h in range(2):
            p = pol_mask[b, h * half:(h + 1) * half, 0]
            cnt = int(p.sum())
            maxcnt = max(maxcnt, cnt, half - cnt)
    npc = NPC_DEFAULT
    while npc * CHUNK < maxcnt:
        npc += SCH
    key = ("nc", nchunks) if npc == NPC_DEFAULT else ("nc", nchunks, npc)
    if key not in _COMPILED:
        _COMPILED[key] = _build(nchunks, npc=npc)
    return _run(_COMPILED[key], flow, event_list, pol_mask, event_mask,
                npc=npc)


# revision 3
# speedup vs baseline: 1.5481x; 1.1122x over previous
"""Trainium2 Bass kernel for the IWE (image-warped-events) problem — v2.

Full inputs in, full outputs out. Data-parallel over (batch, half) across 8
NeuronCores. Host splits each core's 500k events by polarity into two padded
streams (scatter target = pos or neg grid picked per stream, no sign math on
device), packs them in a DMA-friendly superchunk layout, and sums nothing:
each core owns its whole grid; host only stitches batch halves.

Per-core pipeline, per 500-event chunk (125 partitions x 4 blocks):
  - transpose y -> [4,125] (PE array), build dy[bin,e]=y_e-bin via K=1
    accumulating matmuls, onehotY = is_equal(dy,0)
  - flow gather: onehotY^T @ [fy|fx] -> T1[e,256]; x-select by onehot(x)
    product + blocked free-dim reduction
  - my = (1-ts)*fy, mx likewise; staging [my,y,1,mx,x,1] per block,
    one PE transpose; dy2[e,bin] = w_e - bin via K=3 matmuls per block/side
  - hats: y-side Abs+Relu(1-|.|) on scalar engine; x-side negated hat via
    fused abs_max/min on vector engine; scatter = PSUM-accumulated matmuls
    hatY^T @ (-hatX) into the stream's grid; finalize negates.
"""
import numpy as np

H, W = 128, 128
NCORES = 8
CHUNK = 500                            # events per chunk
PBLK = 125                             # partition rows used for events
NBLK = 4                               # blocks per chunk (PBLK*NBLK = CHUNK)
SCH = 8                                # chunks per superchunk (one DMA)
NPC_DEFAULT = 504                      # chunks per polarity stream (>= 252k ev)

_COMPILED = {}


def _build(nchunks, passes=1, npc=NPC_DEFAULT, sch=SCH):
    import concourse.bass as bass
    import concourse.bacc as bacc
    import concourse.mybir as mybir
    from concourse.tile import TileContext

    fp32 = mybir.dt.float32
    fp16 = mybir.dt.float16
    int32 = mybir.dt.int32
    Alu = mybir.AluOpType

    assert npc % sch == 0
    nsc = npc // sch                   # superchunks per stream

    nc = bacc.Bacc("TRN2", target_bir_lowering=False, debug=False,
                   num_devices=NCORES)

    # host-packed events: [2 streams * nsc, 125, sch*16] fully contiguous
    evp = nc.dram_tensor("evp", [2 * nsc, PBLK, sch * 16], fp32,
                         kind="ExternalInput").ap()
    flow = nc.dram_tensor("flow", [2, H, W], fp32, kind="ExternalInput").ap()
    emask = nc.dram_tensor("emask", [H, W], fp32, kind="ExternalInput").ap()
    out = nc.dram_tensor("out", [4, H, W], fp32, kind="ExternalOutput").ap()

    evp_v = evp.rearrange("s p f -> p s f")

    with TileContext(nc) as tc:
        with tc.tile_pool(name="const", bufs=1) as cpool, \
             tc.tile_pool(name="work", bufs=4) as wpool, \
             tc.tile_pool(name="ppool", bufs=1, space="PSUM") as ppool:

            # ---------------- constants ----------------
            iotai = cpool.tile([128, 128], int32)
            nc.gpsimd.iota(iotai[:], pattern=[[1, 128]], base=0,
                           channel_multiplier=0)
            iota32 = cpool.tile([128, 128], fp32)
            nc.vector.tensor_copy(out=iota32[:], in_=iotai[:])
            pidx = cpool.tile([128, 1], int32)
            nc.gpsimd.iota(pidx[:], pattern=[[1, 1]], base=0,
                           channel_multiplier=1)
            ident = cpool.tile([128, 128], fp32)
            nc.vector.tensor_tensor(out=ident[:], in0=iotai[:],
                                    in1=pidx[:].broadcast_to([128, 128]),
                                    op=Alu.is_equal)
            pidxf = cpool.tile([128, 1], fp32)
            nc.vector.tensor_copy(out=pidxf[:], in_=pidx[:])

            def mask_tile(nrows, ones_rows, iota_row, tag):
                # [nrows,128] fp16: selected rows 1.0, iota_row = -iota, rest 0
                ind = cpool.tile([nrows, 1], fp32, tag=tag + "_i")
                tmp = cpool.tile([nrows, 1], fp32, tag=tag + "_t")
                nc.vector.memset(ind[:], 0.0)
                for r in ones_rows:
                    nc.vector.tensor_scalar(out=tmp[:], in0=pidxf[0:nrows],
                                            scalar1=float(r), scalar2=None,
                                            op0=Alu.is_equal)
                    nc.vector.tensor_tensor(out=ind[:], in0=ind[:],
                                            in1=tmp[:], op=Alu.add)
                neg = cpool.tile([nrows, 1], fp32, tag=tag + "_n")
                nc.vector.tensor_scalar(out=neg[:], in0=pidxf[0:nrows],
                                        scalar1=float(iota_row), scalar2=-1.0,
                                        op0=Alu.is_equal, op1=Alu.mult)
                t = cpool.tile([nrows, 128], fp16, tag=tag)
                nc.vector.scalar_tensor_tensor(
                    out=t[:], in0=iota32[0:nrows, :], scalar=neg[:],
                    in1=ind[:].broadcast_to([nrows, 128]),
                    op0=Alu.mult, op1=Alu.add)
                return t

            # dy lhsT masks: dylhs[j] [5,128]: row j = 1, row 4 = -iota
            dylhs = [mask_tile(5, [j], 4, f"dylhs{j}") for j in range(NBLK)]
            # dy2 rhs masks per block [24,256]: cols 0:128 y-side,
            # 128:256 x-side; staging field order per block: [my,y,1,mx,x,1]
            d2rhs = []
            for j in range(NBLK):
                hy_m = mask_tile(24, [6 * j, 6 * j + 1], 6 * j + 2,
                                 f"d2y{j}")
                hx_m = mask_tile(24, [6 * j + 3, 6 * j + 4], 6 * j + 5,
                                 f"d2x{j}")
                m = cpool.tile([24, 256], fp16, tag=f"d2m{j}")
                nc.vector.tensor_copy(out=m[:, 0:128], in_=hy_m[:])
                nc.vector.tensor_copy(out=m[:, 128:256], in_=hx_m[:])
                d2rhs.append(m)

            # flow rows: flow32 natural order [flow0|flow1];
            # flowcat fp16 [fy|fx] = [flow1|flow0]
            flow32 = cpool.tile([128, 256], fp32)
            nc.sync.dma_start(out=flow32[:, 0:128], in_=flow[0])
            nc.sync.dma_start(out=flow32[:, 128:256], in_=flow[1])
            flowcat = cpool.tile([128, 256], fp16)
            nc.vector.tensor_copy(out=flowcat[:, 0:128], in_=flow32[:, 128:256])
            nc.vector.tensor_copy(out=flowcat[:, 128:256], in_=flow32[:, 0:128])

            iota_rep = iota32[:PBLK, :].unsqueeze(1).broadcast_to(
                [PBLK, NBLK, 128])

            # ---------------- psum tiles ----------------
            p_tr1 = ppool.tile([5, PBLK], fp32, tag="tr1")       # transpose1
            p_tr2 = ppool.tile([24, PBLK], fp32, tag="tr2")      # transpose2
            p_dy = ppool.tile([128, CHUNK], fp32, tag="dy")      # [bin, e]
            p_t1 = ppool.tile([PBLK, NBLK * 256], fp32, tag="t1")
            p_d2 = ppool.tile([PBLK, NBLK * 256], fp32, tag="d2")
            p_acc = ppool.tile([128, 128], fp32, tag="acc")
            res = cpool.tile([128, 128 * 4], fp32)

            def phase_a(evt16):
                ev3 = evt16.rearrange("p (n f) -> p n f", f=4)
                ts4 = ev3[:, :, 0]
                y4 = ev3[:, :, 1]
                x4 = ev3[:, :, 2]

                u4 = wpool.tile([PBLK, NBLK], fp32, tag="u4")
                nc.scalar.activation(out=u4[:], in_=ts4,
                                     func=mybir.ActivationFunctionType.Identity,
                                     bias=1.0, scale=-1.0)

                # transpose1: [y|1] -> [5, 125]; st_y5 fp16
                stg_y = wpool.tile([PBLK, 5], fp32, tag="stg_y")
                nc.gpsimd.tensor_copy(out=stg_y[:, 0:4], in_=y4)
                nc.gpsimd.memset(stg_y[:, 4:5], 1.0)
                nc.tensor.transpose(p_tr1[:], stg_y[:],
                                    ident[0:PBLK, 0:PBLK])
                st_y5 = wpool.tile([5, PBLK], fp16, tag="st_y5")
                nc.scalar.copy(out=st_y5[:], in_=p_tr1[:])

                # dy[bin, e] = y_e - bin (K=5, mask lhsT per block)
                for j in range(NBLK):
                    nc.tensor.matmul(
                        out=p_dy[:, j * PBLK:(j + 1) * PBLK],
                        lhsT=dylhs[j][:], rhs=st_y5[:],
                        start=True, stop=True)

                onehotY = wpool.tile([128, CHUNK], fp16, tag="ohY")
                nc.vector.tensor_scalar(out=onehotY[:], in0=p_dy[:],
                                        scalar1=0.0, scalar2=None,
                                        op0=Alu.is_equal)

                # flow row gather: T1[e, fy|fx] per block
                for j in range(NBLK):
                    nc.tensor.matmul(
                        out=p_t1[:, j * 256:(j + 1) * 256],
                        lhsT=onehotY[:, j * PBLK:(j + 1) * PBLK],
                        rhs=flowcat[:], start=True, stop=True)

                # x-select: onehot(x) product + blocked reduce
                ohX = wpool.tile([PBLK, NBLK * 128], fp16, tag="ohX")
                x4_bc = x4.unsqueeze(2).broadcast_to([PBLK, NBLK, 128])
                nc.vector.tensor_tensor(
                    out=ohX[:].rearrange("p (n f) -> p n f", f=128),
                    in0=iota_rep, in1=x4_bc, op=Alu.is_equal)
                prod = wpool.tile([PBLK, NBLK * 256], fp16, tag="prod")
                ohx_rep = ohX[:].rearrange(
                    "p (n f) -> p n f", f=128).unsqueeze(2).broadcast_to(
                    [PBLK, NBLK, 2, 128])
                nc.vector.tensor_tensor(
                    out=prod[:].rearrange("p (n c f) -> p n c f", c=2, f=128),
                    in0=p_t1[:].rearrange("p (n c f) -> p n c f", c=2, f=128),
                    in1=ohx_rep, op=Alu.mult)
                sel = wpool.tile([PBLK, NBLK * 2], fp32, tag="sel")
                nc.vector.tensor_reduce(
                    out=sel[:],
                    in_=prod[:].rearrange("p (n f) -> p n f", f=128),
                    axis=mybir.AxisListType.X, op=Alu.add)
                sel3 = sel[:].rearrange("p (n f) -> p n f", f=2)

                # staging [my, y, 1, mx, x, 1] per block (feeds phase_b)
                stg = wpool.tile([PBLK, NBLK * 6], fp32, tag="stg")
                stg3 = stg[:].rearrange("p (n k) -> p n k", k=6)
                nc.gpsimd.tensor_copy(out=stg3[:, :, 1], in_=y4)
                nc.gpsimd.tensor_copy(out=stg3[:, :, 4], in_=x4)
                ones_v = stg[:].rearrange("p (a b) -> p a b", b=3)[:, :, 2]
                nc.gpsimd.memset(ones_v, 1.0)
                nc.vector.tensor_tensor(out=stg3[:, :, 0], in0=u4[:],
                                        in1=sel3[:, :, 0], op=Alu.mult)
                nc.vector.tensor_tensor(out=stg3[:, :, 3], in0=u4[:],
                                        in1=sel3[:, :, 1], op=Alu.mult)
                return stg

            def phase_b(stg, acc, start=False, stop=False):
                nc.tensor.transpose(p_tr2[:], stg[:],
                                    ident[0:PBLK, 0:PBLK])
                st_w = wpool.tile([24, PBLK], fp16, tag="st_w")
                nc.scalar.copy(out=st_w[:], in_=p_tr2[:])

                # dy2[e, (y-bin | x-bin)] per block (K=24, mask rhs)
                for j in range(NBLK):
                    nc.tensor.matmul(
                        out=p_d2[:, j * 256:(j + 1) * 256],
                        lhsT=st_w[:], rhs=d2rhs[j][:],
                        start=True, stop=True)

                # negated hats both sides: h = min(|dy2|,1) - 1
                ad = wpool.tile([PBLK, NBLK * 256], fp16, tag="ad")
                nc.scalar.activation(out=ad[:], in_=p_d2[:],
                                     func=mybir.ActivationFunctionType.Abs)
                hn = wpool.tile([PBLK, NBLK * 256], fp16, tag="hn")
                nc.scalar.activation(out=hn[:], in_=ad[:],
                                     func=mybir.ActivationFunctionType.Relu,
                                     bias=1.0, scale=-1.0)

                # scatter: acc += hatY (x) hatX  ((-a)(-b) = ab)
                for j in range(NBLK):
                    nc.tensor.matmul(
                        out=acc,
                        lhsT=hn[:, j * 256:j * 256 + 128],
                        rhs=hn[:, j * 256 + 128:(j + 1) * 256],
                        start=(start and j == 0),
                        stop=(stop and j == NBLK - 1))

            def sc_body(i, acc, first=False, last=False):
                evsc = wpool.tile([PBLK, sch * 16], fp32, tag="evsc")
                nc.sync.dma_start(out=evsc[:], in_=evp_v[:, bass.ds(i, 1), :])
                st = phase_a(evsc[:, 0:16])
                for k in range(1, sch):
                    st_next = phase_a(evsc[:, k * 16:(k + 1) * 16])
                    phase_b(st, acc, start=(first and k == 1))
                    st = st_next
                phase_b(st, acc, stop=last)

            for _ in range(passes):
                for s in range(2):
                    lo = s * nsc
                    sc_body(lo, p_acc[:], first=True, last=(nsc == 1))
                    if nsc > 2:
                        tc.For_i_unrolled(lo + 1, lo + nsc - 1, 1,
                                          lambda i: sc_body(i, p_acc[:]),
                                          max_unroll=1)
                    if nsc > 1:
                        sc_body(lo + nsc - 1, p_acc[:], last=True)
                    nc.vector.tensor_copy(out=res[:, s * 128:(s + 1) * 128],
                                          in_=p_acc[:])

            # ---------------- finalize ----------------
            maskt = cpool.tile([128, 128], fp32)
            nc.sync.dma_start(out=maskt[:], in_=emask[:, :])
            inv = 1.0 / (1.0 + 1e-9)
            nc.vector.scalar_tensor_tensor(out=res[:, 256:384],
                                           in0=flow32[:, 0:128], scalar=inv,
                                           in1=maskt[:], op0=Alu.mult,
                                           op1=Alu.mult)
            nc.vector.scalar_tensor_tensor(out=res[:, 384:512],
                                           in0=flow32[:, 128:256], scalar=inv,
                                           in1=maskt[:], op0=Alu.mult,
                                           op1=Alu.mult)
            for ch in range(4):
                nc.sync.dma_start(out=out[ch],
                                  in_=res[:, ch * 128:(ch + 1) * 128])

    nc.compile()
    return nc


def _pack_stream(ev, npc, sch):
    """ev [n, 4] one polarity's events -> [npc//sch, 125, sch*16] fp32."""
    total = npc * CHUNK
    pad = np.zeros((total - ev.shape[0], 4), np.float32)
    if pad.shape[0]:
        pad[:, 0] = 1.0
        pad[:, 1] = -512.0
        pad[:, 2] = -512.0
        ev = np.concatenate([ev, pad], 0)
    nsc = npc // sch
    a = ev.reshape(nsc, sch, PBLK, NBLK * 4)
    return np.ascontiguousarray(a.transpose(0, 2, 1, 3)).reshape(
        nsc, PBLK, sch * 16)


def _make_in_maps(flow, event_list, pol_mask, event_mask, npc=NPC_DEFAULT,
                  sch=SCH):
    half = event_list.shape[1] // 2
    in_maps = []
    for c in range(NCORES):
        b, h = c // 2, c % 2
        ev = event_list[b, h * half:(h + 1) * half, :]
        pos = pol_mask[b, h * half:(h + 1) * half, 0] == 1.0
        packed = np.concatenate([
            _pack_stream(np.ascontiguousarray(ev[pos], np.float32), npc, sch),
            _pack_stream(np.ascontiguousarray(ev[~pos], np.float32), npc, sch),
        ], 0)
        in_maps.append({
            "evp": packed,
            "flow": np.ascontiguousarray(flow[b], np.float32),
            "emask": np.ascontiguousarray(event_mask[b, 0], np.float32),
        })
    return in_maps


def _run(nc, flow, event_list, pol_mask, event_mask, npc=NPC_DEFAULT, sch=SCH):
    from concourse.bass_utils import run_bass_kernel_spmd

    Bb = event_list.shape[0]
    in_maps = _make_in_maps(flow, event_list, pol_mask, event_mask, npc, sch)
    res = run_bass_kernel_spmd(nc, in_maps, list(range(NCORES)))
    out = np.zeros((Bb, 4, H, W), np.float32)
    for c in range(NCORES):
        b = c // 2
        r = res.results[c]["out"]
        out[b, 0:2] += r[0:2]
        if c % 2 == 0:
            out[b, 2:4] = r[2:4]
    return out


def kernel(flow, event_list, pol_mask, event_mask):
    flow = np.asarray(flow, np.float32)
    event_list = np.asarray(event_list, np.float32)
    pol_mask = np.asarray(pol_mask, np.float32)
    event_mask = np.asarray(event_mask, np.float32)
    nchunks = event_list.shape[0] * event_list.shape[1] // NCORES // CHUNK
    # stream capacity: grow npc if a polarity stream overflows the default
    half = event_list.shape[1] // 2
    maxcnt = 0
    for b in range(event_list.shape[0]):
        for h in range(2):
            p = pol_mask[b, h * half:(h + 1) * half, 0]
            cnt = int(p.sum())
            maxcnt = max(maxcnt, cnt, half - cnt)
    npc = NPC_DEFAULT
    while npc * CHUNK < maxcnt:
        npc += SCH
    key = ("nc", nchunks) if npc == NPC_DEFAULT else ("nc", nchunks, npc)
    if key not in _COMPILED:
        _COMPILED[key] = _build(nchunks, npc=npc)
    return _run(_COMPILED[key], flow, event_list, pol_mask, event_mask,
                npc=npc)


# revision 4
# speedup vs baseline: 2.0430x; 1.3197x over previous
"""Trainium2 Bass kernel for the IWE (image-warped-events) problem — v2.

Full inputs in, full outputs out. Data-parallel over (batch, half) across 8
NeuronCores. Host splits each core's 500k events by polarity into two padded
streams (scatter target = pos or neg grid picked per stream, no sign math on
device), packs them in a DMA-friendly superchunk layout, and sums nothing:
each core owns its whole grid; host only stitches batch halves.

Per-core pipeline, per 500-event chunk (125 partitions x 4 blocks):
  - transpose y -> [4,125] (PE array), build dy[bin,e]=y_e-bin via K=1
    accumulating matmuls, onehotY = is_equal(dy,0)
  - flow gather: onehotY^T @ [fy|fx] -> T1[e,256]; x-select by onehot(x)
    product + blocked free-dim reduction
  - my = (1-ts)*fy, mx likewise; staging [my,y,1,mx,x,1] per block,
    one PE transpose; dy2[e,bin] = w_e - bin via K=3 matmuls per block/side
  - hats: y-side Abs+Relu(1-|.|) on scalar engine; x-side negated hat via
    fused abs_max/min on vector engine; scatter = PSUM-accumulated matmuls
    hatY^T @ (-hatX) into the stream's grid; finalize negates.
"""
import numpy as np

H, W = 128, 128
NCORES = 8
CHUNK = 500                            # events per chunk
PBLK = 125                             # partition rows used for events
NBLK = 4                               # blocks per chunk (PBLK*NBLK = CHUNK)
SCH = 8                                # chunks per superchunk (one DMA)
NPC_DEFAULT = 504                      # chunks per polarity stream (>= 252k ev)

_COMPILED = {}


def _build(nchunks, passes=1, npc=NPC_DEFAULT, sch=SCH):
    import concourse.bass as bass
    import concourse.bacc as bacc
    import concourse.mybir as mybir
    from concourse.tile import TileContext

    fp32 = mybir.dt.float32
    fp16 = mybir.dt.float16
    int32 = mybir.dt.int32
    Alu = mybir.AluOpType

    assert npc % sch == 0
    nsc = npc // sch                   # superchunks per stream

    nc = bacc.Bacc("TRN2", target_bir_lowering=False, debug=False,
                   num_devices=NCORES)

    # host-packed events: [2 streams * nsc, 125, sch*16] fully contiguous
    evp = nc.dram_tensor("evp", [2 * nsc, PBLK, sch * 16], fp32,
                         kind="ExternalInput").ap()
    flow = nc.dram_tensor("flow", [2, H, W], fp32, kind="ExternalInput").ap()
    emask = nc.dram_tensor("emask", [H, W], fp32, kind="ExternalInput").ap()
    out = nc.dram_tensor("out", [4, H, W], fp32, kind="ExternalOutput").ap()

    evp_v = evp.rearrange("s p f -> p s f")

    with TileContext(nc) as tc:
        with tc.tile_pool(name="const", bufs=1) as cpool, \
             tc.tile_pool(name="work", bufs=4) as wpool, \
             tc.tile_pool(name="ppool", bufs=1, space="PSUM") as ppool:

            # ---------------- constants ----------------
            iotai = cpool.tile([128, 128], int32)
            nc.gpsimd.iota(iotai[:], pattern=[[1, 128]], base=0,
                           channel_multiplier=0)
            iota32 = cpool.tile([128, 128], fp32)
            nc.vector.tensor_copy(out=iota32[:], in_=iotai[:])
            pidx = cpool.tile([128, 1], int32)
            nc.gpsimd.iota(pidx[:], pattern=[[1, 1]], base=0,
                           channel_multiplier=1)
            ident = cpool.tile([128, 128], fp32)
            nc.vector.tensor_tensor(out=ident[:], in0=iotai[:],
                                    in1=pidx[:].broadcast_to([128, 128]),
                                    op=Alu.is_equal)
            pidxf = cpool.tile([128, 1], fp32)
            nc.vector.tensor_copy(out=pidxf[:], in_=pidx[:])

            def mask_tile(nrows, ones_rows, iota_row, tag):
                # [nrows,128] fp16: selected rows 1.0, iota_row = -iota, rest 0
                ind = cpool.tile([nrows, 1], fp32, tag=tag + "_i")
                tmp = cpool.tile([nrows, 1], fp32, tag=tag + "_t")
                nc.vector.memset(ind[:], 0.0)
                for r in ones_rows:
                    nc.vector.tensor_scalar(out=tmp[:], in0=pidxf[0:nrows],
                                            scalar1=float(r), scalar2=None,
                                            op0=Alu.is_equal)
                    nc.vector.tensor_tensor(out=ind[:], in0=ind[:],
                                            in1=tmp[:], op=Alu.add)
                neg = cpool.tile([nrows, 1], fp32, tag=tag + "_n")
                nc.vector.tensor_scalar(out=neg[:], in0=pidxf[0:nrows],
                                        scalar1=float(iota_row), scalar2=-1.0,
                                        op0=Alu.is_equal, op1=Alu.mult)
                t = cpool.tile([nrows, 128], fp16, tag=tag)
                nc.vector.scalar_tensor_tensor(
                    out=t[:], in0=iota32[0:nrows, :], scalar=neg[:],
                    in1=ind[:].broadcast_to([nrows, 128]),
                    op0=Alu.mult, op1=Alu.add)
                return t

            # dy lhsT masks: dylhs[j] [5,128]: row j = 1, row 4 = -iota
            dylhs = [mask_tile(5, [j], 4, f"dylhs{j}") for j in range(NBLK)]
            # dy2 rhs masks per block [24,256]: cols 0:128 y-side,
            # 128:256 x-side; staging field order per block: [my,y,1,mx,x,1]
            d2rhs = []
            for j in range(NBLK):
                hy_m = mask_tile(24, [6 * j, 6 * j + 1], 6 * j + 2,
                                 f"d2y{j}")
                hx_m = mask_tile(24, [6 * j + 3, 6 * j + 4], 6 * j + 5,
                                 f"d2x{j}")
                m = cpool.tile([24, 256], fp16, tag=f"d2m{j}")
                nc.vector.tensor_copy(out=m[:, 0:128], in_=hy_m[:])
                nc.vector.tensor_copy(out=m[:, 128:256], in_=hx_m[:])
                d2rhs.append(m)

            # flow rows: flow32 natural order [flow0|flow1];
            # flowcat fp16 [fy|fx] = [flow1|flow0]
            flow32 = cpool.tile([128, 256], fp32)
            nc.sync.dma_start(out=flow32[:, 0:128], in_=flow[0])
            nc.sync.dma_start(out=flow32[:, 128:256], in_=flow[1])
            flowcat = cpool.tile([128, 256], fp16)
            nc.vector.tensor_copy(out=flowcat[:, 0:128], in_=flow32[:, 128:256])
            nc.vector.tensor_copy(out=flowcat[:, 128:256], in_=flow32[:, 0:128])

            iota_rep = iota32[:PBLK, :].unsqueeze(1).broadcast_to(
                [PBLK, NBLK, 128])

            # ---------------- psum tiles ----------------
            p_tr1 = ppool.tile([5, PBLK], fp32, tag="tr1")       # transpose1
            p_tr2 = ppool.tile([24, PBLK], fp32, tag="tr2")      # transpose2
            p_dy = ppool.tile([128, CHUNK], fp32, tag="dy")      # [bin, e]
            p_t1 = ppool.tile([PBLK, NBLK * 256], fp32, tag="t1")
            p_d2 = ppool.tile([PBLK, NBLK * 256], fp32, tag="d2")
            p_acc = ppool.tile([128, 128], fp32, tag="acc")
            res = cpool.tile([128, 128 * 4], fp32)

            def phase_a(evt16):
                ev3 = evt16.rearrange("p (n f) -> p n f", f=4)
                ts4 = ev3[:, :, 0]
                y4 = ev3[:, :, 1]
                x4 = ev3[:, :, 2]

                u4 = wpool.tile([PBLK, NBLK], fp32, tag="u4")
                nc.scalar.activation(out=u4[:], in_=ts4,
                                     func=mybir.ActivationFunctionType.Identity,
                                     bias=1.0, scale=-1.0)

                # transpose1: [y|1] -> [5, 125]; st_y5 fp16
                stg_y = wpool.tile([PBLK, 5], fp32, tag="stg_y")
                nc.gpsimd.tensor_copy(out=stg_y[:, 0:4], in_=y4)
                nc.gpsimd.memset(stg_y[:, 4:5], 1.0)
                nc.tensor.transpose(p_tr1[:], stg_y[:],
                                    ident[0:PBLK, 0:PBLK])
                st_y5 = wpool.tile([5, PBLK], fp16, tag="st_y5")
                nc.scalar.copy(out=st_y5[:], in_=p_tr1[:])

                # dy[bin, e] = y_e - bin (K=5, mask lhsT per block)
                for j in range(NBLK):
                    nc.tensor.matmul(
                        out=p_dy[:, j * PBLK:(j + 1) * PBLK],
                        lhsT=dylhs[j][:], rhs=st_y5[:],
                        start=True, stop=True)

                onehotY = wpool.tile([128, CHUNK], fp16, tag="ohY")
                nc.vector.tensor_scalar(out=onehotY[:], in0=p_dy[:],
                                        scalar1=0.0, scalar2=None,
                                        op0=Alu.is_equal)

                # flow row gather: T1[e, fy|fx] per block
                for j in range(NBLK):
                    nc.tensor.matmul(
                        out=p_t1[:, j * 256:(j + 1) * 256],
                        lhsT=onehotY[:, j * PBLK:(j + 1) * PBLK],
                        rhs=flowcat[:], start=True, stop=True)

                # x-select: onehot(x) product + blocked reduce
                ohX = wpool.tile([PBLK, NBLK * 128], fp16, tag="ohX")
                x4_bc = x4.unsqueeze(2).broadcast_to([PBLK, NBLK, 128])
                nc.vector.tensor_tensor(
                    out=ohX[:].rearrange("p (n f) -> p n f", f=128),
                    in0=iota_rep, in1=x4_bc, op=Alu.is_equal)
                prod = wpool.tile([PBLK, NBLK * 256], fp16, tag="prod")
                ohx_rep = ohX[:].rearrange(
                    "p (n f) -> p n f", f=128).unsqueeze(2).broadcast_to(
                    [PBLK, NBLK, 2, 128])
                nc.vector.tensor_tensor(
                    out=prod[:].rearrange("p (n c f) -> p n c f", c=2, f=128),
                    in0=p_t1[:].rearrange("p (n c f) -> p n c f", c=2, f=128),
                    in1=ohx_rep, op=Alu.mult)
                sel = wpool.tile([PBLK, NBLK * 2], fp32, tag="sel")
                nc.vector.tensor_reduce(
                    out=sel[:],
                    in_=prod[:].rearrange("p (n f) -> p n f", f=128),
                    axis=mybir.AxisListType.X, op=Alu.add)
                sel3 = sel[:].rearrange("p (n f) -> p n f", f=2)

                # staging [my, y, 1, mx, x, 1] per block (feeds phase_b)
                stg = wpool.tile([PBLK, NBLK * 6], fp32, tag="stg")
                stg3 = stg[:].rearrange("p (n k) -> p n k", k=6)
                nc.gpsimd.tensor_copy(out=stg3[:, :, 1], in_=y4)
                nc.gpsimd.tensor_copy(out=stg3[:, :, 4], in_=x4)
                ones_v = stg[:].rearrange("p (a b) -> p a b", b=3)[:, :, 2]
                nc.gpsimd.memset(ones_v, 1.0)
                nc.vector.tensor_tensor(out=stg3[:, :, 0], in0=u4[:],
                                        in1=sel3[:, :, 0], op=Alu.mult)
                nc.vector.tensor_tensor(out=stg3[:, :, 3], in0=u4[:],
                                        in1=sel3[:, :, 1], op=Alu.mult)
                return stg

            def phase_b(stg, acc, start=False, stop=False):
                nc.tensor.transpose(p_tr2[:], stg[:],
                                    ident[0:PBLK, 0:PBLK])
                st_w = wpool.tile([24, PBLK], fp16, tag="st_w")
                nc.scalar.copy(out=st_w[:], in_=p_tr2[:])

                # dy2[e, (y-bin | x-bin)] per block (K=24, mask rhs)
                for j in range(NBLK):
                    nc.tensor.matmul(
                        out=p_d2[:, j * 256:(j + 1) * 256],
                        lhsT=st_w[:], rhs=d2rhs[j][:],
                        start=True, stop=True)

                # negated hats both sides: h = min(|dy2|,1) - 1
                ad = wpool.tile([PBLK, NBLK * 256], fp16, tag="ad")
                nc.scalar.activation(out=ad[:], in_=p_d2[:],
                                     func=mybir.ActivationFunctionType.Abs)
                hn = wpool.tile([PBLK, NBLK * 256], fp16, tag="hn")
                nc.scalar.activation(out=hn[:], in_=ad[:],
                                     func=mybir.ActivationFunctionType.Relu,
                                     bias=1.0, scale=-1.0)

                # scatter: acc += hatY (x) hatX  ((-a)(-b) = ab)
                for j in range(NBLK):
                    nc.tensor.matmul(
                        out=acc,
                        lhsT=hn[:, j * 256:j * 256 + 128],
                        rhs=hn[:, j * 256 + 128:(j + 1) * 256],
                        start=(start and j == 0),
                        stop=(stop and j == NBLK - 1))

            def sc_body(i, acc, first=False, last=False):
                evsc = wpool.tile([PBLK, sch * 16], fp32, tag="evsc")
                nc.sync.dma_start(out=evsc[:], in_=evp_v[:, bass.ds(i, 1), :])
                st = phase_a(evsc[:, 0:16])
                for k in range(1, sch):
                    st_next = phase_a(evsc[:, k * 16:(k + 1) * 16])
                    phase_b(st, acc, start=(first and k == 1))
                    st = st_next
                phase_b(st, acc, stop=last)

            for _ in range(passes):
                for s in range(2):
                    lo = s * nsc
                    sc_body(lo, p_acc[:], first=True, last=(nsc == 1))
                    if nsc > 2:
                        tc.For_i_unrolled(lo + 1, lo + nsc - 1, 1,
                                          lambda i: sc_body(i, p_acc[:]),
                                          max_unroll=2)
                    if nsc > 1:
                        sc_body(lo + nsc - 1, p_acc[:], last=True)
                    nc.vector.tensor_copy(out=res[:, s * 128:(s + 1) * 128],
                                          in_=p_acc[:])

            # ---------------- finalize ----------------
            maskt = cpool.tile([128, 128], fp32)
            nc.sync.dma_start(out=maskt[:], in_=emask[:, :])
            inv = 1.0 / (1.0 + 1e-9)
            nc.vector.scalar_tensor_tensor(out=res[:, 256:384],
                                           in0=flow32[:, 0:128], scalar=inv,
                                           in1=maskt[:], op0=Alu.mult,
                                           op1=Alu.mult)
            nc.vector.scalar_tensor_tensor(out=res[:, 384:512],
                                           in0=flow32[:, 128:256], scalar=inv,
                                           in1=maskt[:], op0=Alu.mult,
                                           op1=Alu.mult)
            for ch in range(4):
                nc.sync.dma_start(out=out[ch],
                                  in_=res[:, ch * 128:(ch + 1) * 128])

    nc.compile()
    return nc


def _pack_stream(ev, npc, sch):
    """ev [n, 4] one polarity's events -> [npc//sch, 125, sch*16] fp32."""
    total = npc * CHUNK
    pad = np.zeros((total - ev.shape[0], 4), np.float32)
    if pad.shape[0]:
        pad[:, 0] = 1.0
        pad[:, 1] = -512.0
        pad[:, 2] = -512.0
        ev = np.concatenate([ev, pad], 0)
    nsc = npc // sch
    a = ev.reshape(nsc, sch, PBLK, NBLK * 4)
    return np.ascontiguousarray(a.transpose(0, 2, 1, 3)).reshape(
        nsc, PBLK, sch * 16)


def _make_in_maps(flow, event_list, pol_mask, event_mask, npc=NPC_DEFAULT,
                  sch=SCH):
    half = event_list.shape[1] // 2
    in_maps = []
    for c in range(NCORES):
        b, h = c // 2, c % 2
        ev = event_list[b, h * half:(h + 1) * half, :]
        pos = pol_mask[b, h * half:(h + 1) * half, 0] == 1.0
        packed = np.concatenate([
            _pack_stream(np.ascontiguousarray(ev[pos], np.float32), npc, sch),
            _pack_stream(np.ascontiguousarray(ev[~pos], np.float32), npc, sch),
        ], 0)
        in_maps.append({
            "evp": packed,
            "flow": np.ascontiguousarray(flow[b], np.float32),
            "emask": np.ascontiguousarray(event_mask[b, 0], np.float32),
        })
    return in_maps


def _run(nc, flow, event_list, pol_mask, event_mask, npc=NPC_DEFAULT, sch=SCH):
    from concourse.bass_utils import run_bass_kernel_spmd

    Bb = event_list.shape[0]
    in_maps = _make_in_maps(flow, event_list, pol_mask, event_mask, npc, sch)
    res = run_bass_kernel_spmd(nc, in_maps, list(range(NCORES)))
    out = np.zeros((Bb, 4, H, W), np.float32)
    for c in range(NCORES):
        b = c // 2
        r = res.results[c]["out"]
        out[b, 0:2] += r[0:2]
        if c % 2 == 0:
            out[b, 2:4] = r[2:4]
    return out


def kernel(flow, event_list, pol_mask, event_mask):
    flow = np.asarray(flow, np.float32)
    event_list = np.asarray(event_list, np.float32)
    pol_mask = np.asarray(pol_mask, np.float32)
    event_mask = np.asarray(event_mask, np.float32)
    nchunks = event_list.shape[0] * event_list.shape[1] // NCORES // CHUNK
    # stream capacity: grow npc if a polarity stream overflows the default
    half = event_list.shape[1] // 2
    maxcnt = 0
    for b in range(event_list.shape[0]):
        for h in range(2):
            p = pol_mask[b, h * half:(h + 1) * half, 0]
            cnt = int(p.sum())
            maxcnt = max(maxcnt, cnt, half - cnt)
    npc = NPC_DEFAULT
    while npc * CHUNK < maxcnt:
        npc += SCH
    key = ("nc", nchunks) if npc == NPC_DEFAULT else ("nc", nchunks, npc)
    if key not in _COMPILED:
        _COMPILED[key] = _build(nchunks, npc=npc)
    return _run(_COMPILED[key], flow, event_list, pol_mask, event_mask,
                npc=npc)
